# revision 37
# baseline (speedup 1.0000x reference)
"""Trainium2 Bass kernel for nn_AutoregressiveFlowLayer.

Computes, for batch x [B, D] and R ragged regions (padded to RMAX):
    xg   = x[:, idx] * valid                       [B, R, RMAX]
    h1   = relu(xg @ (W1*M1))                      [B, R, 128]
    h2   = relu(h1 @ (W2*M2))                      [B, R, 128]
    out  = h2 @ (Wout*Mout) -> (shift, log_s)      [B, R, RMAX, 2]
    u    = (xg - shift) * exp(-log_s)
    ll   = sum(valid * (-0.5 u^2 - 0.5 log(2pi) - log_s), -1)   [B, R, 1]

Sharding: data-parallel over batch across 8 NeuronCores; weights replicated.
idx/valid are baked into the compiled program (recompiled if they change).

Device mapping (per core, B_core = 1024):
  - features on partitions, batch on the free axis
  - dma_gather pulls the 32*32 ragged rows of x^T into SBUF, 4 regions
    ("group") packed per 128 partitions
  - L1: 4 row-tiled (K=32) float32r matmuls per group (concurrent row groups)
  - L2: dense [128,128] bf16 matmuls
  - L3: col-tiled (M=32) matmuls -> PSUM [128, B] with shift / logs packed
    per-region on partition strips; relu/exp/elementwise split ACT+DVE
  - partition reduction via matmul with block-diag(-valid) lhsT; the
    Sum(v*log_s) term is folded in as extra accumulating matmuls with
    host-precomputed (Wlogs @ v) vectors; the -0.5*log(2pi)*size constant is
    added by the ACT bias on the final PSUM->SBUF copy.
"""

import os
import sys

import numpy as np

_TRN_REPO = "/opt/trn_rl_repo"
if _TRN_REPO not in sys.path:
    sys.path.insert(0, _TRN_REPO)

D = 1024
R = 32
RMAX = 32
H1 = 128
H2 = 128
B = 8192
NCORES = 8
BC = B // NCORES          # batch per core
NG = R // 4               # 8 groups of 4 regions
BH = 512                  # batch half-tile (one PSUM bank of fp32)
LN2PI = float(np.log(2.0 * np.pi))
EXP_BIAS = float(-0.5 * np.log(2.0))  # exp(-logs + b) = exp(-logs)/sqrt(2)

_cache = {}


def _made_masks_np(idx, valid):
    # nothing to do: masks are passed in as inputs; helper kept for clarity
    pass


def _build_program(idx, valid):
    import concourse.bass as bass
    import concourse.mybir as mybir
    import concourse.tile as tile
    from concourse import bacc

    dt = mybir.dt
    AF = mybir.ActivationFunctionType

    nc = bacc.Bacc("TRN2", target_bir_lowering=False, debug=False)

    # ---- DRAM tensors (per-core inputs) ----
    xT = nc.dram_tensor("xT", [D, BC], dt.bfloat16, kind="ExternalInput").ap()
    w1 = nc.dram_tensor("w1", [128, NG, 128], dt.bfloat16, kind="ExternalInput").ap()
    w2 = nc.dram_tensor("w2", [128, R, 128], dt.bfloat16, kind="ExternalInput").ap()
    w3 = nc.dram_tensor("w3", [128, R, 64], dt.bfloat16, kind="ExternalInput").ap()
    negv = nc.dram_tensor("negv", [128, NG, 4], dt.bfloat16, kind="ExternalInput").ap()
    cb = nc.dram_tensor("cb", [4, NG], dt.float32, kind="ExternalInput").ap()
    idxs_d = nc.dram_tensor("idxs", [128, NG * 8], dt.int16, kind="ExternalInput").ap()
    out_d = nc.dram_tensor("out", [4, NG * BC], dt.float32, kind="ExternalOutput").ap()

    from contextlib import ExitStack

    with tile.TileContext(nc) as tc, ExitStack() as ctx:
        singles = ctx.enter_context(tc.tile_pool(name="singles", bufs=1))
        hs = ctx.enter_context(tc.tile_pool(name="hs", bufs=12))
        es = ctx.enter_context(tc.tile_pool(name="es", bufs=4))
        # PSUM: php = 4x single-bank wave slabs (one region's L1 or L2 out),
        # pssh/pslg = 2x single-bank slabs each for shift / logs (the 4x512
        # ll block is accumulated into the shift bank once d consumed it)
        # -> 8 banks total, fine-grained turnover for deep pipelining.
        php = ctx.enter_context(tc.tile_pool(name="php", bufs=4, space="PSUM"))
        pssh = ctx.enter_context(tc.tile_pool(name="pssh", bufs=2, space="PSUM"))
        pslg = ctx.enter_context(tc.tile_pool(name="pslg", bufs=2, space="PSUM"))

        # ---- load constants into SBUF ----
        w1s = singles.tile([128, NG, 128], dt.bfloat16)
        w2s = singles.tile([128, R, 128], dt.bfloat16)
        w3s = singles.tile([128, R, 64], dt.bfloat16)
        negvs = singles.tile([128, NG, 4], dt.bfloat16)
        cbs = singles.tile([4, NG], dt.float32)
        idxs_s = singles.tile([128, NG * 8], dt.int16)
        # idxs first: the gathers' descriptor generation waits on it, and
        # everything else waits on the first gather. Weights not needed until
        # L3 go after the gathers so their SDMA traffic doesn't starve them.
        nc.sync.dma_start(out=idxs_s[:], in_=idxs_d)
        nc.sync.dma_start(out=w1s[:], in_=w1)
        nc.sync.dma_start(out=w2s[:], in_=w2)

        # gathered ragged inputs (bf16): one tile per group so compute on
        # group g only waits for gather g. One shared count register keeps
        # the Q7 from burning ~0.4us per gather on register MOVEs.
        nreg = nc.gpsimd.to_reg(128)
        xgb = []
        for g in range(NG):
            t = singles.tile([128, 1, BC], dt.bfloat16, tag=f"xgb{g}")
            nc.gpsimd.dma_gather(
                out_ap=t[:],
                in_ap=xT,
                idxs_ap=idxs_s[:, 8 * g:8 * (g + 1)],
                num_idxs=128,
                num_idxs_reg=nreg,
                elem_size=BC,
            )
            xgb.append(t)

        nc.sync.dma_start(out=w3s[:], in_=w3)
        nc.sync.dma_start(out=negvs[:], in_=negv)
        nc.sync.dma_start(out=cbs[:], in_=cb)

        # final output accumulators, split so the first half can DMA out
        # while the second half is still computing
        lls0 = singles.tile([4, NG * BC // 2], dt.float32, tag="lls0")
        lls1 = singles.tile([4, NG * BC // 2], dt.float32, tag="lls1")
        lls01 = [lls0, lls1]

        # per-partition constant bias for the exp
        ebias = singles.tile([128, 1], dt.float32)
        nc.vector.memset(ebias[:], EXP_BIAS)

        nh = BC // BH  # halves per core

        def emit_reduce(prev, on_act):
            # reduce + copy-out for a finished tile: ll4 = -(v.q) - (v.logs)
            # accumulated into the (already consumed) shift bank, then the
            # per-region constant is added by the bias on the PSUM->SBUF copy
            shslab, qt, lgs, g, b0 = prev
            half = NG * BC // 2
            off = g * BC + b0
            lls = lls01[off // half]
            off = off % half
            llp = shslab[0:4, 0:BH]
            nc.tensor.matmul(
                out=llp, lhsT=negvs[:, g, :], rhs=qt[:],
                start=True, stop=False, tile_position=(0, 0),
            )
            nc.tensor.matmul(
                out=llp, lhsT=negvs[:, g, :], rhs=lgs[:],
                start=False, stop=True, tile_position=(0, 0),
            )
            dst = lls[0:4, off: off + BH]
            if on_act:
                nc.scalar.activation(dst, llp, AF.Identity,
                                     bias=cbs[:, g:g + 1], scale=1.0)
            else:
                nc.vector.tensor_scalar_add(dst, llp, cbs[:, g:g + 1])

        prev = None
        step = 0
        for g in range(NG):
            for h in range(nh):
                b0 = h * BH
                xgbs = xgb[g][:, 0, b0:b0 + BH]

                # relu engine pattern across the 8 waves (4 ACT / 4 DVE)
                RELU_ACT = (True, False, True, False, True, False, True, False)

                def relu(widx, dst, src):
                    if RELU_ACT[widx]:
                        nc.scalar.activation(dst, src, AF.Relu)
                    else:
                        nc.vector.tensor_scalar_max(dst, src, 0.0)

                # ---- L1: one row-tiled K=32 bf16 matmul per region wave
                h1sb = []
                for j in range(4):
                    slab = php.tile([128, BH], dt.float32, tag="ph")
                    nc.tensor.matmul(
                        out=slab[:],
                        lhsT=w1s[32 * j:32 * (j + 1), g, :],
                        rhs=xgbs[32 * j:32 * (j + 1), :],
                        start=True, stop=True,
                        tile_position=(32 * j, 0),
                    )
                    h = hs.tile([128, BH], dt.bfloat16, tag="hsb")
                    relu(j, h[:], slab[:])
                    h1sb.append(h)

                # ---- L2: dense K=128 bf16 matmul per region wave
                h2sb = []
                for j in range(4):
                    slab = php.tile([128, BH], dt.float32, tag="ph")
                    nc.tensor.matmul(
                        out=slab[:],
                        lhsT=w2s[:, 4 * g + j, :],
                        rhs=h1sb[j][:],
                        start=True, stop=True,
                        tile_position=(0, 0),
                    )
                    h = hs.tile([128, BH], dt.bfloat16, tag="hsb")
                    relu(4 + j, h[:], slab[:])
                    h2sb.append(h)

                # ---- L3: col-tiled M=32 matmuls into shift / logs banks.
                # All shift matmuls first so d can start while logs compute.
                shsl = pssh.tile([128, BH], dt.float32, tag="sh")
                lgsl = pslg.tile([128, BH], dt.float32, tag="lg")
                for j in range(4):
                    nc.tensor.matmul(
                        out=shsl[32 * j:32 * (j + 1), :],
                        lhsT=w3s[:, 4 * g + j, 0:32],
                        rhs=h2sb[j][:],
                        start=True, stop=True,
                        tile_position=(0, 32 * j),
                    )
                for j in range(4):
                    nc.tensor.matmul(
                        out=lgsl[32 * j:32 * (j + 1), :],
                        lhsT=w3s[:, 4 * g + j, 32:64],
                        rhs=h2sb[j][:],
                        start=True, stop=True,
                        tile_position=(0, 32 * j),
                    )

                # d = xg - shift  (DVE, PSUM operand)
                dtl = es.tile([128, BH], dt.bfloat16, tag="dt")
                nc.vector.tensor_sub(dtl[:], xgbs, shsl[:])
                # E' = exp(-logs)/sqrt(2)  (ACT)
                et = es.tile([128, BH], dt.bfloat16, tag="et")
                nc.scalar.activation(et[:], lgsl[:], AF.Exp,
                                     bias=ebias[:], scale=-1.0)
                # u' = d * E'   ;  q = u'^2 = 0.5 u^2
                ut = es.tile([128, BH], dt.bfloat16, tag="ut")
                nc.vector.tensor_mul(ut[:], dtl[:], et[:])
                qt = es.tile([128, BH], dt.bfloat16, tag="qt")
                nc.vector.tensor_mul(qt[:], ut[:], ut[:])
                # logs copy for next-tile reduce (off the critical path)
                lgs = es.tile([128, BH], dt.bfloat16, tag="lgs")
                if step % 2 == 0:
                    nc.vector.tensor_copy(lgs[:], lgsl[:])
                else:
                    nc.scalar.copy(lgs[:], lgsl[:])

                # reduce of the PREVIOUS tile (its q is ready by now, so the
                # PE never stalls on this tile's elementwise tail)
                if prev is not None:
                    emit_reduce(prev, on_act=(step % 2 == 1))
                    if prev[3] == NG // 2 - 1 and prev[4] == BC - BH:
                        # first output half complete -> drain it early
                        nc.sync.dma_start(out=out_d[:, 0:NG * BC // 2],
                                          in_=lls01[0][:])
                prev = (shsl, qt, lgs, g, b0)
                step += 1

        emit_reduce(prev, on_act=True)
        nc.sync.dma_start(out=out_d[:, NG * BC // 2:], in_=lls01[1][:])

    nc.compile()
    return nc


def _host_prep(inputs, W1, W2, Wout, idx, valid, M1, M2, Mout):
    import ml_dtypes

    bf16 = ml_dtypes.bfloat16
    f32 = np.float32

    idx = np.asarray(idx)
    valid = np.asarray(valid)
    vf = valid.astype(f32)                                  # [R, RMAX]
    Wm1 = (np.asarray(W1) * np.asarray(M1)).astype(f32)     # [R, 32, 128]
    Wm2 = (np.asarray(W2) * np.asarray(M2)).astype(f32)     # [R, 128, 128]
    Wm3 = (np.asarray(Wout) * np.asarray(Mout)).astype(f32)  # [R, 128, 64]
    Wsh = Wm3[:, :, 0::2]                                   # [R, 128, 32]
    Wlg = Wm3[:, :, 1::2]                                   # [R, 128, 32]

    w1 = np.zeros((128, NG, 128), f32)
    for g in range(NG):
        for j in range(4):
            w1[32 * j:32 * (j + 1), g, :] = Wm1[4 * g + j]
    w1 = w1.astype(bf16)
    w2 = np.ascontiguousarray(Wm2.transpose(1, 0, 2)).astype(bf16)  # [128,R,128]
    w3 = np.concatenate([Wsh, Wlg], axis=2)                 # [R, 128, 64]
    w3 = np.ascontiguousarray(w3.transpose(1, 0, 2)).astype(bf16)   # [128,R,64]

    negv = np.zeros((128, NG, 4), f32)
    cbv = np.zeros((4, NG), f32)
    for g in range(NG):
        for j in range(4):
            r = 4 * g + j
            negv[32 * j:32 * (j + 1), g, j] = -vf[r]
            cbv[j, g] = -0.5 * LN2PI * float(vf[r].sum())
    negv = negv.astype(bf16)

    # gather indices: group g, partition p -> row idx[4g + p//32, p%32]
    rows = np.zeros((NG, 128), np.int64)
    for g in range(NG):
        for p in range(128):
            rows[g, p] = idx[4 * g + p // 32, p % 32]
    # [16, num_idxs//16] block, replicated across the 8 gpsimd cores'
    # 16-partition groups (HW convention; sim reads only partitions 0:16)
    idxs = np.zeros((128, NG * 8), np.int16)
    for g in range(NG):
        for i in range(128):
            s, pp = divmod(i, 16)
            for c in range(8):
                idxs[16 * c + pp, 8 * g + s] = rows[g, i]

    xT = np.ascontiguousarray(np.asarray(inputs, dtype=f32).T).astype(bf16)  # [D, B]

    per_core = []
    for c in range(NCORES):
        per_core.append({
            "xT": np.ascontiguousarray(xT[:, c * BC:(c + 1) * BC]),
            "w1": w1, "w2": w2, "w3": w3,
            "negv": negv, "cb": cbv, "idxs": idxs,
        })
    return per_core


def _get_compiled(idx, valid):
    key = (np.asarray(idx).tobytes(), np.asarray(valid).tobytes())
    if _cache.get("key") != key:
        _cache["key"] = key
        _cache["nc"] = _build_program(np.asarray(idx), np.asarray(valid))
    return _cache["nc"]


def _assemble(results):
    full = np.zeros((B, R), np.float32)
    for c in range(NCORES):
        o = results[c]["out"]                       # [4, NG*BC]
        o = o.reshape(4, NG, BC).transpose(2, 1, 0).reshape(BC, R)
        full[c * BC:(c + 1) * BC] = o
    return full[..., None]


def kernel(inputs, W1, W2, Wout, idx, valid, M1, M2, Mout):
    from concourse import bass_utils

    nc = _get_compiled(idx, valid)
    in_maps = _host_prep(inputs, W1, W2, Wout, idx, valid, M1, M2, Mout)
    res = bass_utils.run_bass_kernel_spmd(nc, in_maps, core_ids=list(range(NCORES)))
    out = _assemble(res.results)
    _cache["last_exec_time_ns"] = res.exec_time_ns
    return out


def kernel_profiled(inputs, W1, W2, Wout, idx, valid, M1, M2, Mout, tmpdir=None):
    """Like kernel() but requests an NTFF trace; returns (out, exec_time_ns)."""
    from concourse import bass_utils

    nc = _get_compiled(idx, valid)
    in_maps = _host_prep(inputs, W1, W2, Wout, idx, valid, M1, M2, Mout)
    res = bass_utils.run_bass_kernel_spmd(
        nc, in_maps, core_ids=list(range(NCORES)), trace=True, tmpdir=tmpdir,
    )
    out = _assemble(res.results)
    return out, res.exec_time_ns


# revision 38
# speedup vs baseline: 1.0130x; 1.0130x over previous
"""Trainium2 Bass kernel for nn_AutoregressiveFlowLayer.

Computes, for batch x [B, D] and R ragged regions (padded to RMAX):
    xg   = x[:, idx] * valid                       [B, R, RMAX]
    h1   = relu(xg @ (W1*M1))                      [B, R, 128]
    h2   = relu(h1 @ (W2*M2))                      [B, R, 128]
    out  = h2 @ (Wout*Mout) -> (shift, log_s)      [B, R, RMAX, 2]
    u    = (xg - shift) * exp(-log_s)
    ll   = sum(valid * (-0.5 u^2 - 0.5 log(2pi) - log_s), -1)   [B, R, 1]

Sharding: data-parallel over batch across 8 NeuronCores; weights replicated.
idx/valid are baked into the compiled program (recompiled if they change).

Device mapping (per core, B_core = 1024):
  - features on partitions, batch on the free axis
  - dma_gather pulls the 32*32 ragged rows of x^T into SBUF, 4 regions
    ("group") packed per 128 partitions
  - L1: 4 row-tiled (K=32) float32r matmuls per group (concurrent row groups)
  - L2: dense [128,128] bf16 matmuls
  - L3: col-tiled (M=32) matmuls -> PSUM [128, B] with shift / logs packed
    per-region on partition strips; relu/exp/elementwise split ACT+DVE
  - partition reduction via matmul with block-diag(-valid) lhsT; the
    Sum(v*log_s) term is folded in as extra accumulating matmuls with
    host-precomputed (Wlogs @ v) vectors; the -0.5*log(2pi)*size constant is
    added by the ACT bias on the final PSUM->SBUF copy.
"""

import os
import sys

import numpy as np

_TRN_REPO = "/opt/trn_rl_repo"
if _TRN_REPO not in sys.path:
    sys.path.insert(0, _TRN_REPO)

D = 1024
R = 32
RMAX = 32
H1 = 128
H2 = 128
B = 8192
NCORES = 8
BC = B // NCORES          # batch per core
NG = R // 4               # 8 groups of 4 regions
BH = 512                  # batch half-tile (one PSUM bank of fp32)
LN2PI = float(np.log(2.0 * np.pi))
EXP_BIAS = float(-0.5 * np.log(2.0))  # exp(-logs + b) = exp(-logs)/sqrt(2)

_cache = {}


def _made_masks_np(idx, valid):
    # nothing to do: masks are passed in as inputs; helper kept for clarity
    pass


def _build_program(idx, valid):
    import concourse.bass as bass
    import concourse.mybir as mybir
    import concourse.tile as tile
    from concourse import bacc

    dt = mybir.dt
    AF = mybir.ActivationFunctionType

    nc = bacc.Bacc("TRN2", target_bir_lowering=False, debug=False)

    # ---- DRAM tensors (per-core inputs) ----
    xT = nc.dram_tensor("xT", [D, BC], dt.bfloat16, kind="ExternalInput").ap()
    w1 = nc.dram_tensor("w1", [128, NG, 128], dt.bfloat16, kind="ExternalInput").ap()
    w2 = nc.dram_tensor("w2", [128, R, 128], dt.bfloat16, kind="ExternalInput").ap()
    w3 = nc.dram_tensor("w3", [128, R, 64], dt.bfloat16, kind="ExternalInput").ap()
    negv = nc.dram_tensor("negv", [128, NG, 4], dt.bfloat16, kind="ExternalInput").ap()
    cb = nc.dram_tensor("cb", [4, NG], dt.float32, kind="ExternalInput").ap()
    idxs_d = nc.dram_tensor("idxs", [128, NG * 8], dt.int16, kind="ExternalInput").ap()
    out_d = nc.dram_tensor("out", [4, NG * BC], dt.float32, kind="ExternalOutput").ap()

    from contextlib import ExitStack

    with tile.TileContext(nc) as tc, ExitStack() as ctx:
        singles = ctx.enter_context(tc.tile_pool(name="singles", bufs=1))
        hs = ctx.enter_context(tc.tile_pool(name="hs", bufs=12))
        es = ctx.enter_context(tc.tile_pool(name="es", bufs=4))
        # PSUM: php = 4x single-bank wave slabs (one region's L1 or L2 out),
        # pssh/pslg = 2x single-bank slabs each for shift / logs (the 4x512
        # ll block is accumulated into the shift bank once d consumed it)
        # -> 8 banks total, fine-grained turnover for deep pipelining.
        php = ctx.enter_context(tc.tile_pool(name="php", bufs=4, space="PSUM"))
        pssh = ctx.enter_context(tc.tile_pool(name="pssh", bufs=2, space="PSUM"))
        pslg = ctx.enter_context(tc.tile_pool(name="pslg", bufs=2, space="PSUM"))

        # ---- load constants into SBUF ----
        w1s = singles.tile([128, NG, 128], dt.bfloat16)
        w2s = singles.tile([128, R, 128], dt.bfloat16)
        w3s = singles.tile([128, R, 64], dt.bfloat16)
        negvs = singles.tile([128, NG, 4], dt.bfloat16)
        cbs = singles.tile([4, NG], dt.float32)
        idxs_s = singles.tile([128, NG * 8], dt.int16)
        # idxs first: the gathers' descriptor generation waits on it, and
        # everything else waits on the first gather. Weights not needed until
        # L3 go after the gathers so their SDMA traffic doesn't starve them.
        nc.sync.dma_start(out=idxs_s[:], in_=idxs_d)
        nc.sync.dma_start(out=w1s[:], in_=w1)
        nc.sync.dma_start(out=w2s[:], in_=w2)

        # gathered ragged inputs (bf16): one tile per group so compute on
        # group g only waits for gather g. One shared count register keeps
        # the Q7 from burning ~0.4us per gather on register MOVEs.
        nreg = nc.gpsimd.to_reg(128)
        xgb = []
        for g in range(NG):
            t = singles.tile([128, 1, BC], dt.bfloat16, tag=f"xgb{g}")
            nc.gpsimd.dma_gather(
                out_ap=t[:],
                in_ap=xT,
                idxs_ap=idxs_s[:, 8 * g:8 * (g + 1)],
                num_idxs=128,
                num_idxs_reg=nreg,
                elem_size=BC,
            )
            xgb.append(t)

        nc.sync.dma_start(out=w3s[:], in_=w3)
        nc.sync.dma_start(out=negvs[:], in_=negv)
        nc.sync.dma_start(out=cbs[:], in_=cb)

        # final output accumulators, split so the first half can DMA out
        # while the second half is still computing
        lls0 = singles.tile([4, NG * BC // 2], dt.float32, tag="lls0")
        lls1 = singles.tile([4, NG * BC // 2], dt.float32, tag="lls1")
        lls01 = [lls0, lls1]

        # per-partition constant bias for the exp
        ebias = singles.tile([128, 1], dt.float32)
        nc.vector.memset(ebias[:], EXP_BIAS)

        nh = BC // BH  # halves per core

        def emit_reduce(prev, on_act):
            # reduce + copy-out for a finished tile: ll4 = -(v.q) - (v.logs)
            # accumulated into the (already consumed) shift bank, then the
            # per-region constant is added by the bias on the PSUM->SBUF copy
            shslab, qt, lgs, g, b0 = prev
            half = NG * BC // 2
            off = g * BC + b0
            lls = lls01[off // half]
            off = off % half
            llp = shslab[0:4, 0:BH]
            nc.tensor.matmul(
                out=llp, lhsT=negvs[:, g, :], rhs=qt[:],
                start=True, stop=False, tile_position=(0, 0),
            )
            nc.tensor.matmul(
                out=llp, lhsT=negvs[:, g, :], rhs=lgs[:],
                start=False, stop=True, tile_position=(0, 0),
            )
            dst = lls[0:4, off: off + BH]
            if on_act:
                nc.scalar.activation(dst, llp, AF.Identity,
                                     bias=cbs[:, g:g + 1], scale=1.0)
            else:
                nc.vector.tensor_scalar_add(dst, llp, cbs[:, g:g + 1])

        prev = None
        step = 0
        for g in range(NG):
            for h in range(nh):
                b0 = h * BH
                xgbs = xgb[g][:, 0, b0:b0 + BH]

                # relu engine pattern across the 8 waves: DVE is the busiest
                # engine in steady state, so give ACT the extra relu on odd
                # tiles (avg 4.5 ACT / 3.5 DVE)
                if step % 2 == 0:
                    RELU_ACT = (True, False, True, False, True, False, True, False)
                else:
                    RELU_ACT = (True, True, False, True, True, False, True, False)

                def relu(widx, dst, src):
                    if RELU_ACT[widx]:
                        nc.scalar.activation(dst, src, AF.Relu)
                    else:
                        nc.vector.tensor_scalar_max(dst, src, 0.0)

                # ---- L1: one row-tiled K=32 bf16 matmul per region wave
                h1sb = []
                for j in range(4):
                    slab = php.tile([128, BH], dt.float32, tag="ph")
                    nc.tensor.matmul(
                        out=slab[:],
                        lhsT=w1s[32 * j:32 * (j + 1), g, :],
                        rhs=xgbs[32 * j:32 * (j + 1), :],
                        start=True, stop=True,
                        tile_position=(32 * j, 0),
                    )
                    h = hs.tile([128, BH], dt.bfloat16, tag="hsb")
                    relu(j, h[:], slab[:])
                    h1sb.append(h)

                # ---- L2: dense K=128 bf16 matmul per region wave
                h2sb = []
                for j in range(4):
                    slab = php.tile([128, BH], dt.float32, tag="ph")
                    nc.tensor.matmul(
                        out=slab[:],
                        lhsT=w2s[:, 4 * g + j, :],
                        rhs=h1sb[j][:],
                        start=True, stop=True,
                        tile_position=(0, 0),
                    )
                    h = hs.tile([128, BH], dt.bfloat16, tag="hsb")
                    relu(4 + j, h[:], slab[:])
                    h2sb.append(h)

                # ---- L3: col-tiled M=32 matmuls into shift / logs banks.
                # All shift matmuls first so d can start while logs compute.
                shsl = pssh.tile([128, BH], dt.float32, tag="sh")
                lgsl = pslg.tile([128, BH], dt.float32, tag="lg")
                for j in range(4):
                    nc.tensor.matmul(
                        out=shsl[32 * j:32 * (j + 1), :],
                        lhsT=w3s[:, 4 * g + j, 0:32],
                        rhs=h2sb[j][:],
                        start=True, stop=True,
                        tile_position=(0, 32 * j),
                    )
                for j in range(4):
                    nc.tensor.matmul(
                        out=lgsl[32 * j:32 * (j + 1), :],
                        lhsT=w3s[:, 4 * g + j, 32:64],
                        rhs=h2sb[j][:],
                        start=True, stop=True,
                        tile_position=(0, 32 * j),
                    )

                # d = xg - shift  (DVE, PSUM operand)
                dtl = es.tile([128, BH], dt.bfloat16, tag="dt")
                nc.vector.tensor_sub(dtl[:], xgbs, shsl[:])
                # E' = exp(-logs)/sqrt(2)  (ACT)
                et = es.tile([128, BH], dt.bfloat16, tag="et")
                nc.scalar.activation(et[:], lgsl[:], AF.Exp,
                                     bias=ebias[:], scale=-1.0)
                # u' = d * E'   ;  q = u'^2 = 0.5 u^2
                ut = es.tile([128, BH], dt.bfloat16, tag="ut")
                nc.vector.tensor_mul(ut[:], dtl[:], et[:])
                qt = es.tile([128, BH], dt.bfloat16, tag="qt")
                nc.vector.tensor_mul(qt[:], ut[:], ut[:])
                # logs copy for next-tile reduce (off the critical path)
                lgs = es.tile([128, BH], dt.bfloat16, tag="lgs")
                if step % 2 == 0:
                    nc.vector.tensor_copy(lgs[:], lgsl[:])
                else:
                    nc.scalar.copy(lgs[:], lgsl[:])

                # reduce of the PREVIOUS tile (its q is ready by now, so the
                # PE never stalls on this tile's elementwise tail)
                if prev is not None:
                    emit_reduce(prev, on_act=(step % 2 == 1))
                    if prev[3] == NG // 2 - 1 and prev[4] == BC - BH:
                        # first output half complete -> drain it early
                        nc.sync.dma_start(out=out_d[:, 0:NG * BC // 2],
                                          in_=lls01[0][:])
                prev = (shsl, qt, lgs, g, b0)
                step += 1

        emit_reduce(prev, on_act=True)
        nc.sync.dma_start(out=out_d[:, NG * BC // 2:], in_=lls01[1][:])

    nc.compile()
    return nc


def _host_prep(inputs, W1, W2, Wout, idx, valid, M1, M2, Mout):
    import ml_dtypes

    bf16 = ml_dtypes.bfloat16
    f32 = np.float32

    idx = np.asarray(idx)
    valid = np.asarray(valid)
    vf = valid.astype(f32)                                  # [R, RMAX]
    Wm1 = (np.asarray(W1) * np.asarray(M1)).astype(f32)     # [R, 32, 128]
    Wm2 = (np.asarray(W2) * np.asarray(M2)).astype(f32)     # [R, 128, 128]
    Wm3 = (np.asarray(Wout) * np.asarray(Mout)).astype(f32)  # [R, 128, 64]
    Wsh = Wm3[:, :, 0::2]                                   # [R, 128, 32]
    Wlg = Wm3[:, :, 1::2]                                   # [R, 128, 32]

    w1 = np.zeros((128, NG, 128), f32)
    for g in range(NG):
        for j in range(4):
            w1[32 * j:32 * (j + 1), g, :] = Wm1[4 * g + j]
    w1 = w1.astype(bf16)
    w2 = np.ascontiguousarray(Wm2.transpose(1, 0, 2)).astype(bf16)  # [128,R,128]
    w3 = np.concatenate([Wsh, Wlg], axis=2)                 # [R, 128, 64]
    w3 = np.ascontiguousarray(w3.transpose(1, 0, 2)).astype(bf16)   # [128,R,64]

    negv = np.zeros((128, NG, 4), f32)
    cbv = np.zeros((4, NG), f32)
    for g in range(NG):
        for j in range(4):
            r = 4 * g + j
            negv[32 * j:32 * (j + 1), g, j] = -vf[r]
            cbv[j, g] = -0.5 * LN2PI * float(vf[r].sum())
    negv = negv.astype(bf16)

    # gather indices: group g, partition p -> row idx[4g + p//32, p%32]
    rows = np.zeros((NG, 128), np.int64)
    for g in range(NG):
        for p in range(128):
            rows[g, p] = idx[4 * g + p // 32, p % 32]
    # [16, num_idxs//16] block, replicated across the 8 gpsimd cores'
    # 16-partition groups (HW convention; sim reads only partitions 0:16)
    idxs = np.zeros((128, NG * 8), np.int16)
    for g in range(NG):
        for i in range(128):
            s, pp = divmod(i, 16)
            for c in range(8):
                idxs[16 * c + pp, 8 * g + s] = rows[g, i]

    xT = np.ascontiguousarray(np.asarray(inputs, dtype=f32).T).astype(bf16)  # [D, B]

    per_core = []
    for c in range(NCORES):
        per_core.append({
            "xT": np.ascontiguousarray(xT[:, c * BC:(c + 1) * BC]),
            "w1": w1, "w2": w2, "w3": w3,
            "negv": negv, "cb": cbv, "idxs": idxs,
        })
    return per_core


def _get_compiled(idx, valid):
    key = (np.asarray(idx).tobytes(), np.asarray(valid).tobytes())
    if _cache.get("key") != key:
        _cache["key"] = key
        _cache["nc"] = _build_program(np.asarray(idx), np.asarray(valid))
    return _cache["nc"]


def _assemble(results):
    full = np.zeros((B, R), np.float32)
    for c in range(NCORES):
        o = results[c]["out"]                       # [4, NG*BC]
        o = o.reshape(4, NG, BC).transpose(2, 1, 0).reshape(BC, R)
        full[c * BC:(c + 1) * BC] = o
    return full[..., None]


def kernel(inputs, W1, W2, Wout, idx, valid, M1, M2, Mout):
    from concourse import bass_utils

    nc = _get_compiled(idx, valid)
    in_maps = _host_prep(inputs, W1, W2, Wout, idx, valid, M1, M2, Mout)
    res = bass_utils.run_bass_kernel_spmd(nc, in_maps, core_ids=list(range(NCORES)))
    out = _assemble(res.results)
    _cache["last_exec_time_ns"] = res.exec_time_ns
    return out


def kernel_profiled(inputs, W1, W2, Wout, idx, valid, M1, M2, Mout, tmpdir=None):
    """Like kernel() but requests an NTFF trace; returns (out, exec_time_ns)."""
    from concourse import bass_utils

    nc = _get_compiled(idx, valid)
    in_maps = _host_prep(inputs, W1, W2, Wout, idx, valid, M1, M2, Mout)
    res = bass_utils.run_bass_kernel_spmd(
        nc, in_maps, core_ids=list(range(NCORES)), trace=True, tmpdir=tmpdir,
    )
    out = _assemble(res.results)
    return out, res.exec_time_ns
